# revision 14
# baseline (speedup 1.0000x reference)
"""LIF spike scan kernel for Trainium2, SPMD over 8 NeuronCores.

Problem: x [B=64, T=8, C=128, H=32, W=32] f32.  Per (b,c,h,w) pixel, scan
over T:  v = tau*u + x_t ; s_t = (v > 1) ; u = v*(v <= 1).  Output spikes
[B, T, C, H, W] f32.

Design: all-int16 scaled domain (host ships q = round(x * 2^12) i16,
threshold 4096 = 1.0).  Two pairs of 2-row groups run sequentially; the
two groups (A, B) of a pair interleave on the DVE queue so every adjacent
DVE op is independent — consecutive ops pipeline (~70ns overlap each)
instead of serializing on the 8-slice DRAIN.  Per step and group:
    v = m + q_t               tensor_tensor  i16 + i16 (in place)  2x_1P
    g = (v <= 4096) * 0.5     tensor_scalar  i16 -> f16 {0,0.5}    4x
    m = v * g                 tensor_tensor  i16 x f16 -> i16      2x_1P
Steps 0-6 are bit-packed into ONE byte per pixel by the otherwise-idle
PE (pack += 2^(t+1) * g_t in f32 PSUM; bit t = keep), with each 512-wide
chunk copied to u8 right after its stop-matmul at t=6 — fully hidden
under step 7's DVE work.  Step 7 has no mult (no later step), so its
gate plane is shipped raw as f16 instead of packed: this removes all 16
tail matmuls and leaves a ~1MB DMA as the only exposed tail.  The first
input chunk is split across the sync/scalar queues by group so group A's
scan starts as soon as its half lands.  i16 writeback rounds to nearest
even; 2202/67M flipped spikes vs the f32 reference (rel 1.54e-2 < 2e-2).

Sharding: pure batch-parallel across 8 cores, no collectives.
"""

import numpy as np

B, T, C, HW = 64, 8, 128, 32 * 32
N_CORES = 8
B_LOC = B // N_CORES          # 8 batch rows per core
SCALE = 2.0 ** -12
THI = 4096.0                  # threshold in scaled domain
NSG = 2                       # pairs per core
FG = 2 * HW                   # free dim per group (2 rows x 1024 = 2048)
FP = 2 * FG                   # free dim per pair chunk (4096)

_cache = {}


def _build_nc():
    from concourse import bacc, mybir, tile

    op = mybir.AluOpType
    nc = bacc.Bacc(
        "TRN2", target_bir_lowering=False, debug=False, num_devices=N_CORES
    )
    i16, f16, f32 = mybir.dt.int16, mybir.dt.float16, mybir.dt.float32
    u8 = mybir.dt.uint8
    # q pre-shuffled on host to [sg*T + t, c, (g bl hw)]
    x_ext = nc.dram_tensor(
        "x", [NSG * T, C, FP], i16, kind="ExternalInput"
    ).ap()
    # Pack weights: w[:, t*C:(t+1)*C] = 2^(t+1) * I  (f16, exact), t=0..6
    w_ext = nc.dram_tensor("w", [C, (T - 1) * C], f16,
                           kind="ExternalInput").ap()
    # One byte-plane per group: bit t = keep at step t (t=0..6).
    out_ext = nc.dram_tensor(
        "out", [NSG * 2, C, FG], u8, kind="ExternalOutput"
    ).ap()
    # Raw step-7 gates {0, 0.5} f16, one plane per pair.
    g7_ext = nc.dram_tensor(
        "g7", [NSG, C, FP], f16, kind="ExternalOutput"
    ).ap()

    with tile.TileContext(nc) as tc:
        with tc.tile_pool(name="pool", bufs=2) as pool, tc.tile_pool(
            name="psum", bufs=2, space="PSUM"
        ) as ppool:
            wt = pool.tile([C, (T - 1) * C], f16, tag="w", bufs=1)
            xc = {}
            for sg in range(NSG):
                for t in range(T):
                    xc[t] = pool.tile(
                        [C, FP], i16, tag="x", bufs=5, name=f"x{sg}_{t}"
                    )
                    if sg == 0 and t == 0:
                        # split the critical first load by group across two
                        # queues: group A scans as soon as its half lands
                        nc.sync.dma_start(
                            out=xc[t][:, 0:FG], in_=x_ext[0, :, 0:FG]
                        )
                        nc.scalar.dma_start(
                            out=xc[t][:, FG:FP], in_=x_ext[0, :, FG:FP]
                        )
                        nc.scalar.dma_start(out=wt, in_=w_ext)
                    else:
                        # alternate queues so chunk loads pipeline their
                        # per-transfer latencies
                        eng = nc.sync if (sg * T + t) % 2 == 0 else nc.scalar
                        eng.dma_start(out=xc[t], in_=x_ext[sg * T + t])
                pk = [
                    ppool.tile([C, FG], f32, tag="pk", name=f"pk{sg}_{g}")
                    for g in range(2)
                ]
                pu = [
                    pool.tile([C, FG], u8, tag="pu", bufs=4, name=f"pu{sg}_{g}")
                    for g in range(2)
                ]
                sl = [slice(0, FG), slice(FG, FP)]   # group A / B columns
                mt = {}
                for t in range(T):
                    if t > 0:
                        # v = m + q_t (in place over the x slot)
                        for g in range(2):
                            nc.vector.tensor_tensor(
                                out=xc[t][:, sl[g]], in0=mt[g],
                                in1=xc[t][:, sl[g]], op=op.add,
                            )
                    gt = [
                        pool.tile([C, FG], f16, tag="g", bufs=8,
                                  name=f"g{sg}_{t}_{g}")
                        for g in range(2)
                    ]
                    # keep-gate with tau folded in: {0, 0.5} f16 (4x).
                    # Last step: halves, with each raw-gate slice's DMA
                    # issued as soon as it exists (pipelines the tail DMA
                    # latency under the remaining DVE work).
                    for g in range(2):
                        if t < T - 1:
                            nc.vector.tensor_scalar(
                                out=gt[g], in0=xc[t][:, sl[g]],
                                scalar1=THI, scalar2=0.5,
                                op0=op.is_le, op1=op.mult,
                            )
                        else:
                            for j in range(0, FG, FG // 2):
                                nc.vector.tensor_scalar(
                                    out=gt[g][:, j : j + FG // 2],
                                    in0=xc[t][:, g * FG + j :
                                              g * FG + j + FG // 2],
                                    scalar1=THI, scalar2=0.5,
                                    op0=op.is_le, op1=op.mult,
                                )
                                eng = nc.sync if (2 * g + j // 1024) % 2 == 0 \
                                    else nc.scalar
                                eng.dma_start(
                                    out=g7_ext[sg, :, g * FG + j :
                                               g * FG + j + FG // 2],
                                    in_=gt[g][:, j : j + FG // 2],
                                )
                    if t < T - 1:
                        for g in range(2):
                            mt[g] = pool.tile([C, FG], i16, tag="m", bufs=4,
                                              name=f"m{sg}_{t}_{g}")
                            # m = v * g  (reset + tau; i16 x f16, 2x_1P)
                            nc.vector.tensor_tensor(
                                out=mt[g], in0=xc[t][:, sl[g]], in1=gt[g],
                                op=op.mult,
                            )
                        # pack += 2^(t+1) * g  (PE, f32 PSUM, exact); after
                        # the stop-MM (t=6) copy each chunk out immediately —
                        # all of it hides under step 7's DVE work
                        for g in range(2):
                            for j in range(0, FG, 512):
                                nc.tensor.matmul(
                                    pk[g][:, j : j + 512],
                                    wt[:, t * C : (t + 1) * C],
                                    gt[g][:, j : j + 512],
                                    start=(t == 0),
                                    stop=(t == T - 2),
                                )
                                if t == T - 2:
                                    nc.scalar.copy(
                                        out=pu[g][:, j : j + 512],
                                        in_=pk[g][:, j : j + 512],
                                    )
                            if t == T - 2:
                                # byte plane ships as soon as it's complete —
                                # before the tail, not behind the g7 DMAs
                                nc.sync.dma_start(
                                    out=out_ext[sg * 2 + g], in_=pu[g]
                                )
    nc.compile()
    return nc


def _run(x: np.ndarray, trace: bool = False, tmpdir=None):
    from concourse.bass_utils import run_bass_kernel_spmd

    if "nc" not in _cache:
        _cache["nc"] = _build_nc()
    nc = _cache["nc"]
    x = np.asarray(x)
    q = np.clip(np.rint(x * np.float32(1.0 / SCALE)), -32768, 32767).astype(
        np.int16
    )
    # q[b=(sg*4+g*2+bl), t, c, hw] -> [core, (sg t), c, (g bl hw)]
    q7 = q.reshape(N_CORES, NSG, 2, 2, T, C, HW)
    q_shuf = np.ascontiguousarray(q7.transpose(0, 1, 4, 5, 2, 3, 6)).reshape(
        N_CORES, NSG * T, C, FP
    )
    w = np.zeros((C, (T - 1) * C), dtype=np.float16)
    for t in range(T - 1):
        w[np.arange(C), t * C + np.arange(C)] = np.float16(2.0 ** (t + 1))
    in_maps = [{"x": q_shuf[i], "w": w} for i in range(N_CORES)]
    res = run_bass_kernel_spmd(
        nc, in_maps, core_ids=list(range(N_CORES)), trace=trace, tmpdir=tmpdir
    )
    _cache["last_results"] = res
    outs = [res.results[i]["out"] for i in range(N_CORES)]
    g7s = [res.results[i]["g7"] for i in range(N_CORES)]
    # bytes [core, (sg g), c, (bl hw)]; bit t = keep at step t, t=0..6
    by = np.stack(outs, axis=0).reshape(N_CORES, NSG, 2, 1, C, 2, HW)
    by = by.astype(np.uint8)
    tl = np.arange(T - 1, dtype=np.uint8).reshape(1, 1, 1, T - 1, 1, 1, 1)
    keep = (by >> tl) & np.uint8(1)        # [core, sg, g, t, c, bl, hw]
    spk = (1 - keep).astype(np.float32)
    # step-7 spikes from the raw gates: spike = (g7 == 0)
    g7 = np.stack(g7s, axis=0).reshape(N_CORES, NSG, C, 2, 2, HW)
    s7 = (g7 == 0).astype(np.float32).transpose(0, 1, 3, 2, 4, 5)
    s7 = s7.reshape(N_CORES, NSG, 2, 1, C, 2, HW)
    spk = np.concatenate([spk, s7], axis=3)  # [core, sg, g, T, c, bl, hw]
    out = spk.transpose(0, 1, 2, 5, 3, 4, 6).reshape(B, T, C, HW)
    return np.ascontiguousarray(out).reshape(B, T, C, 32, 32)


def kernel(x: np.ndarray) -> np.ndarray:
    return _run(x, trace=False)


# revision 15
# speedup vs baseline: 1.0557x; 1.0557x over previous
"""LIF spike scan kernel for Trainium2, SPMD over 8 NeuronCores.

Problem: x [B=64, T=8, C=128, H=32, W=32] f32.  Per (b,c,h,w) pixel, scan
over T:  v = tau*u + x_t ; s_t = (v > 1) ; u = v*(v <= 1).  Output spikes
[B, T, C, H, W] f32.

Design: all-int16 scaled domain (host ships q = round(x * 2^12) i16,
threshold 4096 = 1.0).  Two pairs of 2-row groups run sequentially; the
two groups (A, B) of a pair interleave on the DVE queue so every adjacent
DVE op is independent — consecutive ops pipeline (~70ns overlap each)
instead of serializing on the 8-slice DRAIN.  Per step and group:
    v = m + q_t               tensor_tensor  i16 + i16 (in place)  2x_1P
    g = (v <= 4096) * 0.5     tensor_scalar  i16 -> f16 {0,0.5}    4x
    m = v * g                 tensor_tensor  i16 x f16 -> i16      2x_1P
Steps 0-6 are bit-packed into ONE byte per pixel by the otherwise-idle
PE (pack += 2^(t+1) * g_t in f32 PSUM; bit t = keep), with each 512-wide
chunk copied to u8 right after its stop-matmul at t=6 — fully hidden
under step 7's DVE work.  Step 7 has no mult (no later step), so its
gate plane is shipped raw as f16 instead of packed: this removes all 16
tail matmuls and leaves a ~1MB DMA as the only exposed tail.  The first
input chunk is split across the sync/scalar queues by group so group A's
scan starts as soon as its half lands.  i16 writeback rounds to nearest
even; 2202/67M flipped spikes vs the f32 reference (rel 1.54e-2 < 2e-2).

Sharding: pure batch-parallel across 8 cores, no collectives.
"""

import numpy as np

B, T, C, HW = 64, 8, 128, 32 * 32
N_CORES = 8
B_LOC = B // N_CORES          # 8 batch rows per core
SCALE = 2.0 ** -12
THI = 4096.0                  # threshold in scaled domain
NSG = 2                       # pairs per core
FG = 2 * HW                   # free dim per group (2 rows x 1024 = 2048)
FP = 2 * FG                   # free dim per pair chunk (4096)

_cache = {}


def _build_nc():
    from concourse import bacc, mybir, tile

    op = mybir.AluOpType
    nc = bacc.Bacc(
        "TRN2", target_bir_lowering=False, debug=False, num_devices=N_CORES
    )
    i16, f16, f32 = mybir.dt.int16, mybir.dt.float16, mybir.dt.float32
    u8 = mybir.dt.uint8
    # q pre-shuffled on host to [sg*T + t, c, (g bl hw)]
    x_ext = nc.dram_tensor(
        "x", [NSG * T, C, FP], i16, kind="ExternalInput"
    ).ap()
    # Pack weights: w[:, t*C:(t+1)*C] = 2^(t+1) * I  (f16, exact), t=0..6
    w_ext = nc.dram_tensor("w", [C, (T - 1) * C], f16,
                           kind="ExternalInput").ap()
    # One byte-plane per group: bit t = keep at step t (t=0..6).
    out_ext = nc.dram_tensor(
        "out", [NSG * 2, C, FG], u8, kind="ExternalOutput"
    ).ap()
    # Raw step-7 gates {0, 0.5} f16, one plane per pair.
    g7_ext = nc.dram_tensor(
        "g7", [NSG, C, FP], f16, kind="ExternalOutput"
    ).ap()

    with tile.TileContext(nc) as tc:
        with tc.tile_pool(name="pool", bufs=2) as pool, tc.tile_pool(
            name="psum", bufs=2, space="PSUM"
        ) as ppool:
            wt = pool.tile([C, (T - 1) * C], f16, tag="w", bufs=1)
            xc = {}
            for sg in range(NSG):
                for t in range(T):
                    xc[t] = pool.tile(
                        [C, FP], i16, tag="x", bufs=5, name=f"x{sg}_{t}"
                    )
                    if sg == 0 and t == 0:
                        # split the critical first load by group across two
                        # queues: group A scans as soon as its half lands
                        nc.sync.dma_start(
                            out=xc[t][:, 0:FG], in_=x_ext[0, :, 0:FG]
                        )
                        nc.scalar.dma_start(
                            out=xc[t][:, FG:FP], in_=x_ext[0, :, FG:FP]
                        )
                        nc.scalar.dma_start(out=wt, in_=w_ext)
                    else:
                        # bulk input stays on the sync queue — its DMA ring
                        # sustains the full load; the scalar ring is slower
                        nc.sync.dma_start(out=xc[t], in_=x_ext[sg * T + t])
                pk = [
                    ppool.tile([C, FG], f32, tag="pk", name=f"pk{sg}_{g}")
                    for g in range(2)
                ]
                pu = [
                    pool.tile([C, FG], u8, tag="pu", bufs=4, name=f"pu{sg}_{g}")
                    for g in range(2)
                ]
                sl = [slice(0, FG), slice(FG, FP)]   # group A / B columns
                mt = {}
                for t in range(T):
                    if t > 0:
                        # v = m + q_t (in place over the x slot)
                        for g in range(2):
                            nc.vector.tensor_tensor(
                                out=xc[t][:, sl[g]], in0=mt[g],
                                in1=xc[t][:, sl[g]], op=op.add,
                            )
                    gt = [
                        pool.tile([C, FG], f16, tag="g", bufs=8,
                                  name=f"g{sg}_{t}_{g}")
                        for g in range(2)
                    ]
                    # keep-gate with tau folded in: {0, 0.5} f16 (4x).
                    # Last step: halves, with each raw-gate slice's DMA
                    # issued as soon as it exists (pipelines the tail DMA
                    # latency under the remaining DVE work).
                    for g in range(2):
                        if t < T - 1:
                            nc.vector.tensor_scalar(
                                out=gt[g], in0=xc[t][:, sl[g]],
                                scalar1=THI, scalar2=0.5,
                                op0=op.is_le, op1=op.mult,
                            )
                        else:
                            for j in range(0, FG, FG // 2):
                                nc.vector.tensor_scalar(
                                    out=gt[g][:, j : j + FG // 2],
                                    in0=xc[t][:, g * FG + j :
                                              g * FG + j + FG // 2],
                                    scalar1=THI, scalar2=0.5,
                                    op0=op.is_le, op1=op.mult,
                                )
                                eng = nc.sync if (2 * g + j // 1024) % 2 == 0 \
                                    else nc.scalar
                                eng.dma_start(
                                    out=g7_ext[sg, :, g * FG + j :
                                               g * FG + j + FG // 2],
                                    in_=gt[g][:, j : j + FG // 2],
                                )
                    if t < T - 1:
                        for g in range(2):
                            mt[g] = pool.tile([C, FG], i16, tag="m", bufs=4,
                                              name=f"m{sg}_{t}_{g}")
                            # m = v * g  (reset + tau; i16 x f16, 2x_1P)
                            nc.vector.tensor_tensor(
                                out=mt[g], in0=xc[t][:, sl[g]], in1=gt[g],
                                op=op.mult,
                            )
                        # pack += 2^(t+1) * g  (PE, f32 PSUM, exact); after
                        # the stop-MM (t=6) copy each chunk out immediately —
                        # all of it hides under step 7's DVE work
                        for g in range(2):
                            for j in range(0, FG, 512):
                                nc.tensor.matmul(
                                    pk[g][:, j : j + 512],
                                    wt[:, t * C : (t + 1) * C],
                                    gt[g][:, j : j + 512],
                                    start=(t == 0),
                                    stop=(t == T - 2),
                                )
                                if t == T - 2:
                                    nc.scalar.copy(
                                        out=pu[g][:, j : j + 512],
                                        in_=pk[g][:, j : j + 512],
                                    )
                            if t == T - 2:
                                # byte plane ships as soon as it's complete —
                                # before the tail, not behind the g7 DMAs
                                nc.sync.dma_start(
                                    out=out_ext[sg * 2 + g], in_=pu[g]
                                )
    nc.compile()
    return nc


def _run(x: np.ndarray, trace: bool = False, tmpdir=None):
    from concourse.bass_utils import run_bass_kernel_spmd

    if "nc" not in _cache:
        _cache["nc"] = _build_nc()
    nc = _cache["nc"]
    x = np.asarray(x)
    q = np.clip(np.rint(x * np.float32(1.0 / SCALE)), -32768, 32767).astype(
        np.int16
    )
    # q[b=(sg*4+g*2+bl), t, c, hw] -> [core, (sg t), c, (g bl hw)]
    q7 = q.reshape(N_CORES, NSG, 2, 2, T, C, HW)
    q_shuf = np.ascontiguousarray(q7.transpose(0, 1, 4, 5, 2, 3, 6)).reshape(
        N_CORES, NSG * T, C, FP
    )
    w = np.zeros((C, (T - 1) * C), dtype=np.float16)
    for t in range(T - 1):
        w[np.arange(C), t * C + np.arange(C)] = np.float16(2.0 ** (t + 1))
    in_maps = [{"x": q_shuf[i], "w": w} for i in range(N_CORES)]
    res = run_bass_kernel_spmd(
        nc, in_maps, core_ids=list(range(N_CORES)), trace=trace, tmpdir=tmpdir
    )
    _cache["last_results"] = res
    outs = [res.results[i]["out"] for i in range(N_CORES)]
    g7s = [res.results[i]["g7"] for i in range(N_CORES)]
    # bytes [core, (sg g), c, (bl hw)]; bit t = keep at step t, t=0..6
    by = np.stack(outs, axis=0).reshape(N_CORES, NSG, 2, 1, C, 2, HW)
    by = by.astype(np.uint8)
    tl = np.arange(T - 1, dtype=np.uint8).reshape(1, 1, 1, T - 1, 1, 1, 1)
    keep = (by >> tl) & np.uint8(1)        # [core, sg, g, t, c, bl, hw]
    spk = (1 - keep).astype(np.float32)
    # step-7 spikes from the raw gates: spike = (g7 == 0)
    g7 = np.stack(g7s, axis=0).reshape(N_CORES, NSG, C, 2, 2, HW)
    s7 = (g7 == 0).astype(np.float32).transpose(0, 1, 3, 2, 4, 5)
    s7 = s7.reshape(N_CORES, NSG, 2, 1, C, 2, HW)
    spk = np.concatenate([spk, s7], axis=3)  # [core, sg, g, T, c, bl, hw]
    out = spk.transpose(0, 1, 2, 5, 3, 4, 6).reshape(B, T, C, HW)
    return np.ascontiguousarray(out).reshape(B, T, C, 32, 32)


def kernel(x: np.ndarray) -> np.ndarray:
    return _run(x, trace=False)
